# revision 6
# baseline (speedup 1.0000x reference)
"""Cross-attention Trainium2 kernel (nn_CrossAttention).

Reference computation (per batch b):
    q = Wq @ x1 + bq            [32, N]     (N = 64*64 = 4096)
    k = Wk @ x2 + bk            [32, N]
    v = Wv @ x2 + bv            [256, N]
    attn = softmax(q^T k, axis over keys m)     [N, N]
    out[c, n] = sum_m v[c, m] attn[n, m]        [256, N]

Sharding: 8 cores = 4 batches x 2 query-halves (2048 queries per core, all
4096 keys).  Each core runs the same NEFF on its own input slice; softmax
rows are complete within a core so no cross-core communication is needed.

Per-core kernel layout choices:
  * Inputs x1/x2 and weights are fp16 on the host side: projections run at
    full PE rate (1 cyc/row) with half the HBM traffic of fp32, and fp16's
    11-bit mantissa keeps q/k logits accurate (bf16 inputs push the final
    rel-err past 1.5e-2; fp16 keeps it ~3e-3).
  * S^T tiles [keys m on partitions, queries n on free dim] so the second
    matmul (attn @ V) consumes exp(S^T) directly from SBUF with m as the
    contraction dim -- no transposes anywhere.
  * Q and K are produced replicated 4x across partition groups (Wq/Wk
    stacked 4x on the host) so the D=32-contraction QK^T matmuls can be
    row-packed 4-per-PE-array via tile_position.
  * The S^T PSUM is split into two [128, 1024] halves on a 2-deep pool:
    exp of half h (step i) overlaps the S^T matmuls of step i+1 instead of
    serializing behind them (single-buffer psum was the baseline's pacer).
  * exp(S^T) is written in bf16 by the ACT engine only (exact exp); all
    PSUM evacuation / normalization work runs on DVE + GpSimd so ACT never
    stalls the softmax pipeline.
  * Row-sum partials land on partitions {0,32,64,96} via col-packed M=1
    ones-matmuls; a DMA gather + K=4 ones-matmul combines and broadcasts
    them, then a fast approximate reciprocal normalizes.
  * Softmax skips the max-subtraction: logits are ~N(0, 32), |s| < ~48
    for this problem size, exp() stays comfortably inside fp32/bf16 range.
  * bv is folded in at the end: out += bv (softmax rows sum to 1).
"""

import sys

for _p in (
    "/root/.axon_site",
    "/root/.axon_site/_ro/trn_rl_repo",
    "/root/.axon_site/_ro/pypackages",
):
    if _p not in sys.path:
        sys.path.append(_p)

import numpy as np

import concourse.bass as bass
from concourse import bacc
import concourse.tile as tile
from concourse import mybir
from concourse import bass_utils

B = 4
C = 256          # value/input channels
D = 32           # q/k channels
N = 4096         # keys per batch (64*64)
NQ = 2048        # queries per core (half a batch)
NT = 512         # query tile (free dim of S^T / output matmuls)
NNT = NQ // NT   # 4 query tiles
NSC = 8          # key super-chunks of 512 (4 x 128) keys
F32 = mybir.dt.float32
F32R = mybir.dt.float32r
F16 = mybir.dt.float16
BF16 = mybir.dt.bfloat16
AFT = mybir.ActivationFunctionType


def attn_tile_kernel(tc, out, x1, x2, wq4t, wk4t, wvt, bq4, bk4, bv, ones_c, ones_f):
    nc = tc.nc

    with (
        tc.tile_pool(name="consts", bufs=1) as consts,
        tc.tile_pool(name="bigbuf", bufs=1) as bigbuf,
        # 4 pt bufs: two halves are allocated per step BEFORE the previous
        # step's AV consumers are emitted; with <4 bufs a new exp would
        # reuse a buffer whose reader isn't emitted yet (untracked race).
        tc.tile_pool(name="ptbuf", bufs=4) as ptbuf,
        tc.tile_pool(name="finbuf", bufs=2) as finbuf,
    ):
        # ---- constants / weights -------------------------------------
        ones_rs = consts.tile([128, 32], BF16, name="ones_rs")
        nc.sync.dma_start(out=ones_rs, in_=ones_c)
        ones_bc = consts.tile([128, 128], F32R, name="ones_bc")
        nc.sync.dma_start(out=ones_bc, in_=ones_f)

        bq4_sb = consts.tile([128, 1], F32, name="bq4_sb")
        nc.sync.dma_start(out=bq4_sb, in_=bq4)
        bk4_sb = consts.tile([128, 1], F32, name="bk4_sb")
        nc.sync.dma_start(out=bk4_sb, in_=bk4)
        bv_sb = []
        for cc in range(2):
            t = consts.tile([128, 1], F32, name=f"bv_sb{cc}")
            nc.sync.dma_start(out=t, in_=bv[cc * 128 : (cc + 1) * 128, :])
            bv_sb.append(t)

        wq4t_sb, wk4t_sb, wvt_sb = [], [], []
        for kc in range(2):
            rows = slice(kc * 128, (kc + 1) * 128)
            t = consts.tile([128, 128], F16, name=f"wq4t_sb{kc}")
            nc.sync.dma_start(out=t, in_=wq4t[rows, :])
            wq4t_sb.append(t)
            t = consts.tile([128, 128], F16, name=f"wk4t_sb{kc}")
            nc.scalar.dma_start(out=t, in_=wk4t[rows, :])
            wk4t_sb.append(t)
            t = consts.tile([128, 256], F16, name=f"wvt_sb{kc}")
            nc.scalar.dma_start(out=t, in_=wvt[rows, :])
            wvt_sb.append(t)

        # ---- feature maps (x1 first: Q4 is on the critical path) -----
        x1_sb = [
            bigbuf.tile([128, NQ], F16, name="x1_sb0"),
            bigbuf.tile([128, NQ], F16, name="x1_sb1"),
        ]
        x2_sb = [
            bigbuf.tile([128, N], F16, name="x2_sb0"),
            bigbuf.tile([128, N], F16, name="x2_sb1"),
        ]
        for blk in range(2):
            cols = slice(blk * 1024, (blk + 1) * 1024)
            nc.sync.dma_start(out=x1_sb[0][:, cols], in_=x1[0:128, cols])
            nc.scalar.dma_start(out=x1_sb[1][:, cols], in_=x1[128:256, cols])
        for blk in range(4):
            cols = slice(blk * 1024, (blk + 1) * 1024)
            nc.sync.dma_start(out=x2_sb[0][:, cols], in_=x2[0:128, cols])
            nc.scalar.dma_start(out=x2_sb[1][:, cols], in_=x2[128:256, cols])

        q4_sb = bigbuf.tile([128, NQ], F32R, name="q4_sb")
        k4_sb = bigbuf.tile([128, N], F32R, name="k4_sb")
        vt_sb = bigbuf.tile([128, C * N // 128], BF16, name="vt_sb")  # [128, 8192]

        # ---- prep: projections ---------------------------------------
        # Interleaved per 1024-column x2 block so PE work becomes available
        # as each DMA block lands: Q4 first (x1), then per block K4 + V^T.
        with tc.tile_pool(name="prep_psum", bufs=2, space="PSUM") as pp:
            # Q4 [128, 2048] = (Wq stacked 4x) @ x1, then +bq
            psum_q = pp.tile([128, NQ], F32, name="psum_q", tag="prep")
            for t4 in range(NNT):
                cols = slice(t4 * NT, (t4 + 1) * NT)
                for kc in range(2):
                    nc.tensor.matmul(
                        psum_q[:, cols],
                        lhsT=wq4t_sb[kc],
                        rhs=x1_sb[kc][:, cols],
                        start=(kc == 0),
                        stop=(kc == 1),
                    )
            nc.vector.tensor_scalar_add(q4_sb, psum_q, bq4_sb)

            for blk in range(4):
                bcols = slice(blk * 1024, (blk + 1) * 1024)
                # K4 for this block
                psum_k = pp.tile([128, 1024], F32, name=f"psum_k{blk}", tag="prep")
                for t2 in range(2):
                    cols = slice(t2 * NT, (t2 + 1) * NT)
                    src_c = slice(blk * 1024 + t2 * NT, blk * 1024 + (t2 + 1) * NT)
                    for kc in range(2):
                        nc.tensor.matmul(
                            psum_k[:, cols],
                            lhsT=wk4t_sb[kc],
                            rhs=x2_sb[kc][:, src_c],
                            start=(kc == 0),
                            stop=(kc == 1),
                        )
                nc.vector.tensor_scalar_add(k4_sb[:, bcols], psum_k, bk4_sb)
                # V^T (bf16) for this block's 8 m-chunks
                psum_v = pp.tile([128, 2048], F32, name=f"psum_v{blk}", tag="prep")
                for m8 in range(8):
                    mc = 8 * blk + m8
                    for kc in range(2):
                        nc.tensor.matmul(
                            psum_v[:, m8 * 256 : (m8 + 1) * 256],
                            lhsT=x2_sb[kc][:, mc * 128 : (mc + 1) * 128],
                            rhs=wvt_sb[kc],
                            start=(kc == 0),
                            stop=(kc == 1),
                        )
                for h in range(2):
                    cols = slice(h * 1024, (h + 1) * 1024)
                    dst = vt_sb[:, blk * 2048 + h * 1024 : blk * 2048 + (h + 1) * 1024]
                    if h == 0:
                        nc.scalar.copy(dst, psum_v[:, cols])
                    else:
                        nc.vector.tensor_copy(dst, psum_v[:, cols])

        # ---- main attention loop -------------------------------------
        # Flat software pipeline over (nt, sc) steps, each split into two
        # halves h of 2 key-chunks.  The S^T psum is a 2-deep pool of
        # [128, 1024] halves, so exp (ACT) of half (i, h) overlaps the S^T
        # matmuls of the next half/step on the PE instead of serializing.
        # AV/rowsum matmuls of step i are emitted after step i+1's S^T, so
        # the PE always has work while ACT computes exp.
        with (
            tc.tile_pool(name="s_psum", bufs=2, space="PSUM") as sp,
            tc.tile_pool(name="o_psum", bufs=1, space="PSUM") as op,
            tc.tile_pool(name="b_psum", bufs=1, space="PSUM") as bp,
        ):
            state = {}

            def _emit_st_half(nt, sc, h):
                # S^T half: 2 row-packed matmuls (chunks 4*sc+2h+{0,1}) at
                # row-groups {2h, 2h+1}; exp -> bf16 pt on ACT.
                qcols = slice(nt * NT, (nt + 1) * NT)
                psum_s = sp.tile([128, 2 * NT], F32, name=f"ps_{nt}_{sc}_{h}", tag="s")
                for j in range(2):
                    mc = 4 * sc + 2 * h + j
                    rowg = slice(32 * (2 * h + j), 32 * (2 * h + j + 1))
                    nc.tensor.matmul(
                        psum_s[:, j * NT : (j + 1) * NT],
                        lhsT=k4_sb[rowg, mc * 128 : (mc + 1) * 128],
                        rhs=q4_sb[rowg, qcols],
                        start=True,
                        stop=True,
                        tile_position=(32 * (2 * h + j), 0),
                    )
                pt = ptbuf.tile([128, 2 * NT], BF16, name=f"pt_{nt}_{sc}_{h}", tag="pt")
                nc.scalar.activation(out=pt, in_=psum_s, func=AFT.Exp)
                return pt

            def _emit_av_half(nt, sc, h, pt):
                first, last = (sc == 0 and h == 0), (sc == NSC - 1 and h == 1)
                if first:
                    state[nt] = (
                        op.tile([128, NT], F32, name=f"po0_{nt}", tag="o0"),
                        op.tile([128, NT], F32, name=f"po1_{nt}", tag="o1"),
                        op.tile([128, NT], F32, name=f"prs_{nt}", tag="rs"),
                    )
                psum_o0, psum_o1, psum_rs = state[nt]
                for j in range(2):
                    # col-packed rowsums: M=1 tiles, partials land on
                    # partitions {0, 32, 64, 96} across the two halves.
                    # Each group g is written once per sc, so start/stop key
                    # on sc alone (half b's groups also need start at sc=0).
                    pcols = slice(j * NT, (j + 1) * NT)
                    g = 2 * h + j
                    nc.tensor.matmul(
                        psum_rs[32 * g : 32 * (g + 1), :],
                        lhsT=ones_rs,
                        rhs=pt[:, pcols],
                        start=(sc == 0),
                        stop=(sc == NSC - 1),
                        tile_position=(0, 32 * g),
                        skip_group_check=True,
                    )
                for j in range(2):
                    mc = 4 * sc + 2 * h + j
                    pcols = slice(j * NT, (j + 1) * NT)
                    for cc in range(2):
                        nc.tensor.matmul(
                            (psum_o0, psum_o1)[cc],
                            lhsT=vt_sb[
                                :, mc * 256 + cc * 128 : mc * 256 + (cc + 1) * 128
                            ],
                            rhs=pt[:, pcols],
                            start=(first and j == 0),
                            stop=(last and j == 1),
                        )

            def _emit_fin(nt):
                # evacuate PSUM fast (frees banks for the next tile), then
                # normalize on SBUF; everything off the ACT engine so exp
                # never stalls.
                psum_o0, psum_o1, psum_rs = state.pop(nt)
                qcols = slice(nt * NT, (nt + 1) * NT)
                rs_sb = finbuf.tile([128, NT], F32R, name=f"rs_sb_{nt}", tag="rs_sb")
                nc.vector.tensor_copy(rs_sb, psum_rs)
                # GpSimd has no PSUM port: psum evacuation stays on ACT/DVE
                # (one ACT copy per 8-step tile fits in ACT's slack).
                raw0 = finbuf.tile([128, NT], F32, name=f"raw0_{nt}", tag="raw0")
                nc.scalar.copy(raw0, psum_o0)
                raw1 = finbuf.tile([128, NT], F32, name=f"raw1_{nt}", tag="raw1")
                nc.vector.tensor_copy(raw1, psum_o1)
                # gather the 4 partial rows onto adjacent partitions, then a
                # K=4 ones-matmul combines + broadcasts to all 128 partitions
                rs4p = finbuf.tile([4, NT], F32R, name=f"rs4p_{nt}", tag="rs4p")
                nc.sync.dma_start(out=rs4p, in_=rs_sb[0:97:32, :])
                psum_b = bp.tile([128, NT], F32, name=f"pb_{nt}", tag="b")
                nc.tensor.matmul(
                    psum_b, lhsT=ones_bc[0:4, :], rhs=rs4p, start=True, stop=True
                )
                rbc = finbuf.tile([128, NT], F32, name=f"rbc_{nt}", tag="rbc")
                nc.vector.reciprocal_approx_fast(out=rbc, in_=psum_b)
                for cc, raw in ((0, raw0), (1, raw1)):
                    t_sb = finbuf.tile([128, NT], F32, name=f"t_{nt}_{cc}", tag=f"t{cc}")
                    nc.gpsimd.tensor_mul(t_sb, raw, rbc)
                    o_sb = finbuf.tile([128, NT], F32, name=f"o_{nt}_{cc}", tag=f"o{cc}")
                    nc.gpsimd.tensor_scalar_add(o_sb, t_sb, bv_sb[cc])
                    nc.sync.dma_start(
                        out=out[cc * 128 : (cc + 1) * 128, qcols], in_=o_sb
                    )

            steps = [(nt, sc) for nt in range(NNT) for sc in range(NSC)]
            prev = None
            for nt, sc in steps:
                pt_a = _emit_st_half(nt, sc, 0)
                pt_b = _emit_st_half(nt, sc, 1)
                if prev is not None:
                    pnt, psc, ppa, ppb = prev
                    _emit_av_half(pnt, psc, 0, ppa)
                    _emit_av_half(pnt, psc, 1, ppb)
                    if psc == NSC - 1:
                        _emit_fin(pnt)
                prev = (nt, sc, pt_a, pt_b)
            pnt, psc, ppa, ppb = prev
            _emit_av_half(pnt, psc, 0, ppa)
            _emit_av_half(pnt, psc, 1, ppb)
            _emit_fin(pnt)


def build_nc():
    nc = bacc.Bacc("TRN2", target_bir_lowering=False, debug=False)
    x1 = nc.dram_tensor("x1", [C, NQ], F16, kind="ExternalInput").ap()
    x2 = nc.dram_tensor("x2", [C, N], F16, kind="ExternalInput").ap()
    wq4t = nc.dram_tensor("wq4t", [C, 128], F16, kind="ExternalInput").ap()
    wk4t = nc.dram_tensor("wk4t", [C, 128], F16, kind="ExternalInput").ap()
    wvt = nc.dram_tensor("wvt", [C, C], F16, kind="ExternalInput").ap()
    bq4 = nc.dram_tensor("bq4", [128, 1], F32, kind="ExternalInput").ap()
    bk4 = nc.dram_tensor("bk4", [128, 1], F32, kind="ExternalInput").ap()
    bv = nc.dram_tensor("bv", [C, 1], F32, kind="ExternalInput").ap()
    ones_cd = nc.dram_tensor("ones_c", [128, 32], BF16, kind="ExternalInput").ap()
    ones_fd = nc.dram_tensor("ones_f", [128, 128], F32R, kind="ExternalInput").ap()
    out = nc.dram_tensor("out", [C, NQ], F32, kind="ExternalOutput").ap()
    with tile.TileContext(nc) as tc:
        attn_tile_kernel(
            tc, out, x1, x2, wq4t, wk4t, wvt, bq4, bk4, bv, ones_cd, ones_fd
        )
    nc.compile()
    return nc


def make_in_maps(f1, f2, Wq, bq, Wk, bk, Wv, bv):
    f1 = np.asarray(f1, dtype=np.float32)
    f2 = np.asarray(f2, dtype=np.float32)
    Wq = np.asarray(Wq, dtype=np.float32)
    Wk = np.asarray(Wk, dtype=np.float32)
    Wv = np.asarray(Wv, dtype=np.float32)
    bq = np.asarray(bq, dtype=np.float32)
    bk = np.asarray(bk, dtype=np.float32)
    bv = np.asarray(bv, dtype=np.float32)

    x1 = f1.reshape(B, C, N).astype(np.float16)
    x2 = f2.reshape(B, C, N).astype(np.float16)
    wq4t = np.ascontiguousarray(np.concatenate([Wq.T] * 4, axis=1).astype(np.float16))
    wk4t = np.ascontiguousarray(np.concatenate([Wk.T] * 4, axis=1).astype(np.float16))
    wvt = np.ascontiguousarray(Wv.T.astype(np.float16))                # [256, 256]
    bq4 = np.ascontiguousarray(np.tile(bq, 4).reshape(128, 1))
    bk4 = np.ascontiguousarray(np.tile(bk, 4).reshape(128, 1))
    bvv = np.ascontiguousarray(bv.reshape(C, 1))
    import ml_dtypes

    ones_c = np.ones((128, 32), ml_dtypes.bfloat16)
    ones_f = np.ones((128, 128), np.float32)

    in_maps = []
    for core in range(8):
        b, h = divmod(core, 2)
        in_maps.append(
            dict(
                x1=np.ascontiguousarray(x1[b, :, h * NQ : (h + 1) * NQ]),
                x2=np.ascontiguousarray(x2[b]),
                wq4t=wq4t,
                wk4t=wk4t,
                wvt=wvt,
                bq4=bq4,
                bk4=bk4,
                bv=bvv,
                ones_c=ones_c,
                ones_f=ones_f,
            )
        )
    return in_maps


_NC_CACHE = None


def _get_nc():
    global _NC_CACHE
    if _NC_CACHE is None:
        _NC_CACHE = build_nc()
    return _NC_CACHE


def kernel(f1, f2, Wq, bq, Wk, bk, Wv, bv):
    in_maps = make_in_maps(f1, f2, Wq, bq, Wk, bk, Wv, bv)
    res = bass_utils.run_bass_kernel_spmd(_get_nc(), in_maps, core_ids=list(range(8)))
    out = np.empty((B, C, N), np.float32)
    for core in range(8):
        b, h = divmod(core, 2)
        out[b, :, h * NQ : (h + 1) * NQ] = res.results[core]["out"]
    return out.reshape(B, C, 64, 64)


# revision 13
# speedup vs baseline: 1.0367x; 1.0367x over previous
"""Cross-attention Trainium2 kernel (nn_CrossAttention).

Reference computation (per batch b):
    q = Wq @ x1 + bq            [32, N]     (N = 64*64 = 4096)
    k = Wk @ x2 + bk            [32, N]
    v = Wv @ x2 + bv            [256, N]
    attn = softmax(q^T k, axis over keys m)     [N, N]
    out[c, n] = sum_m v[c, m] attn[n, m]        [256, N]

Sharding: 8 cores = 4 batches x 2 query-halves (2048 queries per core, all
4096 keys).  Each core runs the same NEFF on its own input slice; softmax
rows are complete within a core so no cross-core communication is needed.

Per-core kernel layout choices:
  * Inputs x1/x2 and weights are fp16 on the host side: projections run at
    full PE rate (1 cyc/row) with half the HBM traffic of fp32, and fp16's
    11-bit mantissa keeps q/k logits accurate (bf16 inputs push the final
    rel-err past 1.5e-2; fp16 keeps it ~3e-3).
  * S^T tiles [keys m on partitions, queries n on free dim] so the second
    matmul (attn @ V) consumes exp(S^T) directly from SBUF with m as the
    contraction dim -- no transposes anywhere.
  * Q and K are produced replicated 4x across partition groups (Wq/Wk
    stacked 4x on the host) so the D=32-contraction QK^T matmuls can be
    row-packed 4-per-PE-array via tile_position.
  * The S^T PSUM is split into two [128, 1024] halves on a 2-deep pool:
    exp of half h (step i) overlaps the S^T matmuls of step i+1 instead of
    serializing behind them (single-buffer psum was the baseline's pacer).
  * exp(S^T) is written in bf16 by the ACT engine only (exact exp); all
    PSUM evacuation / normalization work runs on DVE + GpSimd so ACT never
    stalls the softmax pipeline.
  * Row-sum partials land on partitions {0,32,64,96} via col-packed M=1
    ones-matmuls; a DMA gather + K=4 ones-matmul combines and broadcasts
    them, then a fast approximate reciprocal normalizes.
  * Softmax skips the max-subtraction: logits are ~N(0, 32), |s| < ~48
    for this problem size, exp() stays comfortably inside fp32/bf16 range.
  * bv is folded in at the end: out += bv (softmax rows sum to 1).
"""

import sys

for _p in (
    "/root/.axon_site",
    "/root/.axon_site/_ro/trn_rl_repo",
    "/root/.axon_site/_ro/pypackages",
):
    if _p not in sys.path:
        sys.path.append(_p)

import numpy as np

import concourse.bass as bass
from concourse import bacc
import concourse.tile as tile
from concourse import mybir
from concourse import bass_utils

B = 4
C = 256          # value/input channels
D = 32           # q/k channels
N = 4096         # keys per batch (64*64)
NQ = 2048        # queries per core (half a batch)
NT = 512         # query tile (free dim of S^T / output matmuls)
NNT = NQ // NT   # 4 query tiles
NSC = 8          # key super-chunks of 512 (4 x 128) keys
F32 = mybir.dt.float32
F32R = mybir.dt.float32r
F16 = mybir.dt.float16
BF16 = mybir.dt.bfloat16
AFT = mybir.ActivationFunctionType


def attn_tile_kernel(tc, out, x1, x2, wq4t, wk4t, wvt, bq4, bk4, bv, ones_c, ones_f):
    nc = tc.nc

    with (
        tc.tile_pool(name="consts", bufs=1) as consts,
        tc.tile_pool(name="bigbuf", bufs=1) as bigbuf,
        # 4 pt bufs: two halves are allocated per step BEFORE the previous
        # step's AV consumers are emitted; with <4 bufs a new exp would
        # reuse a buffer whose reader isn't emitted yet (untracked race).
        tc.tile_pool(name="ptbuf", bufs=4) as ptbuf,
        tc.tile_pool(name="finbuf", bufs=2) as finbuf,
    ):
        # ---- constants / weights -------------------------------------
        ones_rs = consts.tile([128, 32], BF16, name="ones_rs")
        nc.sync.dma_start(out=ones_rs, in_=ones_c)
        ones_bc = consts.tile([128, 128], F32R, name="ones_bc")
        nc.sync.dma_start(out=ones_bc, in_=ones_f)

        bq4_sb = consts.tile([128, 1], F32, name="bq4_sb")
        nc.sync.dma_start(out=bq4_sb, in_=bq4)
        bk4_sb = consts.tile([128, 1], F32, name="bk4_sb")
        nc.sync.dma_start(out=bk4_sb, in_=bk4)
        bv_sb = []
        for cc in range(2):
            t = consts.tile([128, 1], F32, name=f"bv_sb{cc}")
            nc.sync.dma_start(out=t, in_=bv[cc * 128 : (cc + 1) * 128, :])
            bv_sb.append(t)

        # Weights are host-interleaved to match the channel-pair feature
        # layout: plane ch holds channels {2p+ch}.
        wq4t_sb, wk4t_sb, wvt_sb = [], [], []
        for ch in range(2):
            rows = slice(ch * 128, (ch + 1) * 128)
            t = consts.tile([128, 128], F16, name=f"wq4t_sb{ch}")
            nc.sync.dma_start(out=t, in_=wq4t[rows, :])
            wq4t_sb.append(t)
            t = consts.tile([128, 128], F16, name=f"wk4t_sb{ch}")
            nc.sync.dma_start(out=t, in_=wk4t[rows, :])
            wk4t_sb.append(t)
            t = consts.tile([128, 256], F16, name=f"wvt_sb{ch}")
            nc.sync.dma_start(out=t, in_=wvt[rows, :])
            wvt_sb.append(t)

        # ---- feature maps -------------------------------------------
        # Host passes x1/x2 reshaped [128, 2*cols]: partition p holds the
        # channel pair (2p, 2p+1) back-to-back, so each DMA line is one
        # fully contiguous 4-16KB read (vs 128 small strided descriptors).
        # Spread across 4 DGE queues so the input wall is ~4x shorter.
        x1_sb = bigbuf.tile([128, 2 * NQ], F16, name="x1_sb")
        x2_sb = bigbuf.tile([128, 2 * N], F16, name="x2_sb")
        nc.sync.dma_start(out=x1_sb, in_=x1)
        nc.scalar.dma_start(out=x2_sb[0:64, :], in_=x2[0:64, :])
        nc.gpsimd.dma_start(out=x2_sb[64:128, :], in_=x2[64:128, :])

        def x1p(ch, cols):
            return x1_sb[:, ch * NQ + cols.start : ch * NQ + cols.stop]

        def x2p(ch, cols):
            return x2_sb[:, ch * N + cols.start : ch * N + cols.stop]

        q4_sb = bigbuf.tile([128, NQ], F32R, name="q4_sb")
        k4_sb = bigbuf.tile([128, N], F32R, name="k4_sb")
        vt_sb = bigbuf.tile([128, C * N // 128], BF16, name="vt_sb")  # [128, 8192]

        # ---- prep: projections ---------------------------------------
        # Interleaved per 1024-column x2 block so PE work becomes available
        # as each DMA block lands: Q4 first (x1), then per block K4 + V^T.
        with tc.tile_pool(name="prep_psum", bufs=2, space="PSUM") as pp:
            # Q4 [128, 2048] = (Wq stacked 4x) @ x1, then +bq
            psum_q = pp.tile([128, NQ], F32, name="psum_q", tag="prep")
            for t4 in range(NNT):
                cols = slice(t4 * NT, (t4 + 1) * NT)
                for ch in range(2):
                    nc.tensor.matmul(
                        psum_q[:, cols],
                        lhsT=wq4t_sb[ch],
                        rhs=x1p(ch, cols),
                        start=(ch == 0),
                        stop=(ch == 1),
                    )
            nc.vector.tensor_scalar_add(q4_sb, psum_q, bq4_sb)

            for blk in range(4):
                bcols = slice(blk * 1024, (blk + 1) * 1024)
                # K4 for this block
                psum_k = pp.tile([128, 1024], F32, name=f"psum_k{blk}", tag="prep")
                for t2 in range(2):
                    cols = slice(t2 * NT, (t2 + 1) * NT)
                    src_c = slice(blk * 1024 + t2 * NT, blk * 1024 + (t2 + 1) * NT)
                    for ch in range(2):
                        nc.tensor.matmul(
                            psum_k[:, cols],
                            lhsT=wk4t_sb[ch],
                            rhs=x2p(ch, src_c),
                            start=(ch == 0),
                            stop=(ch == 1),
                        )
                nc.vector.tensor_scalar_add(k4_sb[:, bcols], psum_k, bk4_sb)
                # V^T (bf16) for this block's 8 m-chunks
                psum_v = pp.tile([128, 2048], F32, name=f"psum_v{blk}", tag="prep")
                for m8 in range(8):
                    mc = 8 * blk + m8
                    for ch in range(2):
                        nc.tensor.matmul(
                            psum_v[:, m8 * 256 : (m8 + 1) * 256],
                            lhsT=x2p(ch, slice(mc * 128, (mc + 1) * 128)),
                            rhs=wvt_sb[ch],
                            start=(ch == 0),
                            stop=(ch == 1),
                        )
                for h in range(2):
                    cols = slice(h * 1024, (h + 1) * 1024)
                    dst = vt_sb[:, blk * 2048 + h * 1024 : blk * 2048 + (h + 1) * 1024]
                    if h == 0:
                        nc.scalar.copy(dst, psum_v[:, cols])
                    else:
                        nc.vector.tensor_copy(dst, psum_v[:, cols])

        # ---- main attention loop -------------------------------------
        # Flat software pipeline over (nt, sc) steps, each split into two
        # halves h of 2 key-chunks.  The S^T psum is a 2-deep pool of
        # [128, 1024] halves, so exp (ACT) of half (i, h) overlaps the S^T
        # matmuls of the next half/step on the PE instead of serializing.
        # AV/rowsum matmuls of step i are emitted after step i+1's S^T, so
        # the PE always has work while ACT computes exp.
        with (
            tc.tile_pool(name="s_psum", bufs=2, space="PSUM") as sp,
            tc.tile_pool(name="o_psum", bufs=1, space="PSUM") as op,
            tc.tile_pool(name="b_psum", bufs=1, space="PSUM") as bp,
        ):
            state = {}

            def _emit_st_half(nt, sc, h):
                # S^T half: 2 row-packed matmuls (chunks 4*sc+2h+{0,1}) at
                # row-groups {2h, 2h+1}; exp -> bf16 pt on ACT.
                qcols = slice(nt * NT, (nt + 1) * NT)
                psum_s = sp.tile([128, 2 * NT], F32, name=f"ps_{nt}_{sc}_{h}", tag="s")
                for j in range(2):
                    mc = 4 * sc + 2 * h + j
                    rowg = slice(32 * (2 * h + j), 32 * (2 * h + j + 1))
                    nc.tensor.matmul(
                        psum_s[:, j * NT : (j + 1) * NT],
                        lhsT=k4_sb[rowg, mc * 128 : (mc + 1) * 128],
                        rhs=q4_sb[rowg, qcols],
                        start=True,
                        stop=True,
                        tile_position=(32 * (2 * h + j), 0),
                    )
                pt = ptbuf.tile([128, 2 * NT], BF16, name=f"pt_{nt}_{sc}_{h}", tag="pt")
                nc.scalar.activation(out=pt, in_=psum_s, func=AFT.Exp)
                return pt

            def _emit_av_half(nt, sc, h, pt):
                first, last = (sc == 0 and h == 0), (sc == NSC - 1 and h == 1)
                if first:
                    state[nt] = (
                        op.tile([128, NT], F32, name=f"po0_{nt}", tag="o0"),
                        op.tile([128, NT], F32, name=f"po1_{nt}", tag="o1"),
                        op.tile([128, NT], F32, name=f"prs_{nt}", tag="rs"),
                    )
                psum_o0, psum_o1, psum_rs = state[nt]
                for j in range(2):
                    # col-packed rowsums: M=1 tiles, partials land on
                    # partitions {0, 32, 64, 96} across the two halves.
                    # Each group g is written once per sc, so start/stop key
                    # on sc alone (half b's groups also need start at sc=0).
                    pcols = slice(j * NT, (j + 1) * NT)
                    g = 2 * h + j
                    nc.tensor.matmul(
                        psum_rs[32 * g : 32 * (g + 1), :],
                        lhsT=ones_rs,
                        rhs=pt[:, pcols],
                        start=(sc == 0),
                        stop=(sc == NSC - 1),
                        tile_position=(0, 32 * g),
                        skip_group_check=True,
                    )
                for j in range(2):
                    mc = 4 * sc + 2 * h + j
                    pcols = slice(j * NT, (j + 1) * NT)
                    for cc in range(2):
                        nc.tensor.matmul(
                            (psum_o0, psum_o1)[cc],
                            lhsT=vt_sb[
                                :, mc * 256 + cc * 128 : mc * 256 + (cc + 1) * 128
                            ],
                            rhs=pt[:, pcols],
                            start=(first and j == 0),
                            stop=(last and j == 1),
                        )

            def _emit_fin(nt):
                # evacuate PSUM fast (frees banks for the next tile), then
                # normalize on SBUF; everything off the ACT engine so exp
                # never stalls.
                psum_o0, psum_o1, psum_rs = state.pop(nt)
                qcols = slice(nt * NT, (nt + 1) * NT)
                rs_sb = finbuf.tile([128, NT], F32R, name=f"rs_sb_{nt}", tag="rs_sb")
                nc.vector.tensor_copy(rs_sb, psum_rs)
                # GpSimd has no PSUM port: psum evacuation stays on ACT/DVE
                # (one ACT copy per 8-step tile fits in ACT's slack).
                raw0 = finbuf.tile([128, NT], F32, name=f"raw0_{nt}", tag="raw0")
                nc.scalar.copy(raw0, psum_o0)
                raw1 = finbuf.tile([128, NT], F32, name=f"raw1_{nt}", tag="raw1")
                nc.vector.tensor_copy(raw1, psum_o1)
                # gather the 4 partial rows onto adjacent partitions, then a
                # K=4 ones-matmul combines + broadcasts to all 128 partitions
                rs4p = finbuf.tile([4, NT], F32R, name=f"rs4p_{nt}", tag="rs4p")
                nc.sync.dma_start(out=rs4p, in_=rs_sb[0:97:32, :])
                psum_b = bp.tile([128, NT], F32, name=f"pb_{nt}", tag="b")
                nc.tensor.matmul(
                    psum_b, lhsT=ones_bc[0:4, :], rhs=rs4p, start=True, stop=True
                )
                rbc = finbuf.tile([128, NT], F32, name=f"rbc_{nt}", tag="rbc")
                nc.vector.reciprocal_approx_fast(out=rbc, in_=psum_b)
                # keep element-wise work on DVE: GpSimd's software tensor ops
                # measure ~15x slower than DVE on hardware
                for cc, raw in ((0, raw0), (1, raw1)):
                    t_sb = finbuf.tile([128, NT], F32, name=f"t_{nt}_{cc}", tag=f"t{cc}")
                    nc.vector.tensor_mul(t_sb, raw, rbc)
                    o_sb = finbuf.tile([128, NT], F32, name=f"o_{nt}_{cc}", tag=f"o{cc}")
                    nc.vector.tensor_scalar_add(o_sb, t_sb, bv_sb[cc])
                    nc.sync.dma_start(
                        out=out[cc * 128 : (cc + 1) * 128, qcols], in_=o_sb
                    )

            steps = [(nt, sc) for nt in range(NNT) for sc in range(NSC)]
            prev = None
            for nt, sc in steps:
                pt_a = _emit_st_half(nt, sc, 0)
                pt_b = _emit_st_half(nt, sc, 1)
                if prev is not None:
                    pnt, psc, ppa, ppb = prev
                    _emit_av_half(pnt, psc, 0, ppa)
                    _emit_av_half(pnt, psc, 1, ppb)
                    if psc == NSC - 1:
                        _emit_fin(pnt)
                prev = (nt, sc, pt_a, pt_b)
            pnt, psc, ppa, ppb = prev
            _emit_av_half(pnt, psc, 0, ppa)
            _emit_av_half(pnt, psc, 1, ppb)
            _emit_fin(pnt)


def build_nc():
    nc = bacc.Bacc("TRN2", target_bir_lowering=False, debug=False)
    x1 = nc.dram_tensor("x1", [128, 2 * NQ], F16, kind="ExternalInput").ap()
    x2 = nc.dram_tensor("x2", [128, 2 * N], F16, kind="ExternalInput").ap()
    wq4t = nc.dram_tensor("wq4t", [C, 128], F16, kind="ExternalInput").ap()
    wk4t = nc.dram_tensor("wk4t", [C, 128], F16, kind="ExternalInput").ap()
    wvt = nc.dram_tensor("wvt", [C, C], F16, kind="ExternalInput").ap()
    bq4 = nc.dram_tensor("bq4", [128, 1], F32, kind="ExternalInput").ap()
    bk4 = nc.dram_tensor("bk4", [128, 1], F32, kind="ExternalInput").ap()
    bv = nc.dram_tensor("bv", [C, 1], F32, kind="ExternalInput").ap()
    ones_cd = nc.dram_tensor("ones_c", [128, 32], BF16, kind="ExternalInput").ap()
    ones_fd = nc.dram_tensor("ones_f", [128, 128], F32R, kind="ExternalInput").ap()
    out = nc.dram_tensor("out", [C, NQ], F32, kind="ExternalOutput").ap()
    with tile.TileContext(nc) as tc:
        attn_tile_kernel(
            tc, out, x1, x2, wq4t, wk4t, wvt, bq4, bk4, bv, ones_cd, ones_fd
        )
    nc.compile()
    return nc


def make_in_maps(f1, f2, Wq, bq, Wk, bk, Wv, bv):
    f1 = np.asarray(f1, dtype=np.float32)
    f2 = np.asarray(f2, dtype=np.float32)
    Wq = np.asarray(Wq, dtype=np.float32)
    Wk = np.asarray(Wk, dtype=np.float32)
    Wv = np.asarray(Wv, dtype=np.float32)
    bq = np.asarray(bq, dtype=np.float32)
    bk = np.asarray(bk, dtype=np.float32)
    bv = np.asarray(bv, dtype=np.float32)

    x1 = f1.reshape(B, C, N).astype(np.float16)
    x2 = f2.reshape(B, C, N).astype(np.float16)
    # channel-pair packing: SBUF partition p holds channels (2p, 2p+1), so
    # weight plane ch = rows (2p+ch) of W^T.
    wq4t_f = np.concatenate([Wq.T] * 4, axis=1).astype(np.float16)   # [256, 128]
    wk4t_f = np.concatenate([Wk.T] * 4, axis=1).astype(np.float16)
    wvt_f = Wv.T.astype(np.float16)                                  # [256, 256]
    wq4t = np.ascontiguousarray(np.concatenate([wq4t_f[0::2], wq4t_f[1::2]], axis=0))
    wk4t = np.ascontiguousarray(np.concatenate([wk4t_f[0::2], wk4t_f[1::2]], axis=0))
    wvt = np.ascontiguousarray(np.concatenate([wvt_f[0::2], wvt_f[1::2]], axis=0))
    bq4 = np.ascontiguousarray(np.tile(bq, 4).reshape(128, 1))
    bk4 = np.ascontiguousarray(np.tile(bk, 4).reshape(128, 1))
    bvv = np.ascontiguousarray(bv.reshape(C, 1))
    import ml_dtypes

    ones_c = np.ones((128, 32), ml_dtypes.bfloat16)
    ones_f = np.ones((128, 128), np.float32)

    in_maps = []
    for core in range(8):
        b, h = divmod(core, 2)
        in_maps.append(
            dict(
                # [C, cols] -> [128, 2*cols]: rows (2p, 2p+1) concatenated per
                # partition; a plain reshape since channel rows are adjacent.
                x1=np.ascontiguousarray(x1[b, :, h * NQ : (h + 1) * NQ]).reshape(
                    128, 2 * NQ
                ),
                x2=np.ascontiguousarray(x2[b]).reshape(128, 2 * N),
                wq4t=wq4t,
                wk4t=wk4t,
                wvt=wvt,
                bq4=bq4,
                bk4=bk4,
                bv=bvv,
                ones_c=ones_c,
                ones_f=ones_f,
            )
        )
    return in_maps


_NC_CACHE = None


def _get_nc():
    global _NC_CACHE
    if _NC_CACHE is None:
        _NC_CACHE = build_nc()
    return _NC_CACHE


def kernel(f1, f2, Wq, bq, Wk, bk, Wv, bv):
    in_maps = make_in_maps(f1, f2, Wq, bq, Wk, bk, Wv, bv)
    res = bass_utils.run_bass_kernel_spmd(_get_nc(), in_maps, core_ids=list(range(8)))
    out = np.empty((B, C, N), np.float32)
    for core in range(8):
        b, h = divmod(core, 2)
        out[b, :, h * NQ : (h + 1) * NQ] = res.results[core]["out"]
    return out.reshape(B, C, 64, 64)


# revision 21
# speedup vs baseline: 1.1786x; 1.1369x over previous
"""Cross-attention Trainium2 kernel (nn_CrossAttention).

Reference computation (per batch b):
    q = Wq @ x1 + bq            [32, N]     (N = 64*64 = 4096)
    k = Wk @ x2 + bk            [32, N]
    v = Wv @ x2 + bv            [256, N]
    attn = softmax(q^T k, axis over keys m)     [N, N]
    out[c, n] = sum_m v[c, m] attn[n, m]        [256, N]

Sharding: 8 cores = 4 batches x 2 query-halves (2048 queries per core, all
4096 keys).  Each core runs the same NEFF on its own input slice; softmax
rows are complete within a core so no cross-core communication is needed.

Per-core kernel layout choices:
  * Inputs x1/x2 and weights are fp16 on the host side: projections run at
    full PE rate (1 cyc/row) with half the HBM traffic of fp32, and fp16's
    11-bit mantissa keeps q/k logits accurate (bf16 inputs push the final
    rel-err past 1.5e-2; fp16 keeps it ~3e-3).
  * S^T tiles [keys m on partitions, queries n on free dim] so the second
    matmul (attn @ V) consumes exp(S^T) directly from SBUF with m as the
    contraction dim -- no transposes anywhere.
  * Q and K are produced replicated 4x across partition groups (Wq/Wk
    stacked 4x on the host) so the D=32-contraction QK^T matmuls can be
    row-packed 4-per-PE-array via tile_position.
  * The S^T PSUM is split into two [128, 1024] halves on a 2-deep pool:
    exp of half h (step i) overlaps the S^T matmuls of step i+1 instead of
    serializing behind them (single-buffer psum was the baseline's pacer).
  * exp(S^T) is written in bf16 by the ACT engine only (exact exp); all
    PSUM evacuation / normalization work runs on DVE + GpSimd so ACT never
    stalls the softmax pipeline.
  * Row-sum partials land on partitions {0,32,64,96} via col-packed M=1
    ones-matmuls; a DMA gather + K=4 ones-matmul combines and broadcasts
    them, then a fast approximate reciprocal normalizes.
  * Softmax skips the max-subtraction: logits are ~N(0, 32), |s| < ~48
    for this problem size, exp() stays comfortably inside fp32/bf16 range.
  * bv is folded in at the end: out += bv (softmax rows sum to 1).
"""

import sys

for _p in (
    "/root/.axon_site",
    "/root/.axon_site/_ro/trn_rl_repo",
    "/root/.axon_site/_ro/pypackages",
):
    if _p not in sys.path:
        sys.path.append(_p)

import numpy as np

import concourse.bass as bass
from concourse import bacc
import concourse.tile as tile
from concourse import mybir
from concourse import bass_utils

B = 4
C = 256          # value/input channels
D = 32           # q/k channels
N = 4096         # keys per batch (64*64)
NQ = 2048        # queries per core (half a batch)
NT = 512         # query tile (free dim of S^T / output matmuls)
NNT = NQ // NT   # 4 query tiles
NSC = 8          # key super-chunks of 512 (4 x 128) keys
F32 = mybir.dt.float32
F32R = mybir.dt.float32r
F16 = mybir.dt.float16
BF16 = mybir.dt.bfloat16
AFT = mybir.ActivationFunctionType


def attn_tile_kernel(tc, out, x1, x2, wall, biases, ones_c, ones_f):
    nc = tc.nc

    with (
        tc.tile_pool(name="consts", bufs=1) as consts,
        tc.tile_pool(name="bigbuf", bufs=1) as bigbuf,
        # 4 pt bufs: two halves are allocated per step BEFORE the previous
        # step's AV consumers are emitted; with <4 bufs a new exp would
        # reuse a buffer whose reader isn't emitted yet (untracked race).
        tc.tile_pool(name="ptbuf", bufs=4) as ptbuf,
        tc.tile_pool(name="finbuf", bufs=2) as finbuf,
    ):
        # ---- constants / weights -------------------------------------
        ones_rs = consts.tile([128, 32], BF16, name="ones_rs")
        nc.scalar.dma_start(out=ones_rs, in_=ones_c)
        ones_bc = consts.tile([128, 128], F32R, name="ones_bc")
        nc.scalar.dma_start(out=ones_bc, in_=ones_f)

        # biases packed as one [128, 4] f32: cols = bq4 | bk4 | bv0 | bv1
        bias_sb = consts.tile([128, 4], F32, name="bias_sb")
        nc.scalar.dma_start(out=bias_sb, in_=biases)
        bq4_sb = bias_sb[:, 0:1]
        bk4_sb = bias_sb[:, 1:2]
        bv_sb = [bias_sb[:, 2:3], bias_sb[:, 3:4]]

        # All weights ride in one contiguous [128, 1024] fp16 tensor (one
        # 2KB-per-partition DMA instead of ~800 tiny descriptors), already
        # host-interleaved to the channel-pair layout: plane ch holds
        # channels {2p+ch}, cols = [wq 128 | wk 128 | wv 256] per plane.
        wall_sb = consts.tile([128, 1024], F16, name="wall_sb")
        nc.scalar.dma_start(out=wall_sb, in_=wall)
        wq4t_sb = [wall_sb[:, ch * 512 : ch * 512 + 128] for ch in range(2)]
        wk4t_sb = [wall_sb[:, ch * 512 + 128 : ch * 512 + 256] for ch in range(2)]
        wvt_sb = [wall_sb[:, ch * 512 + 256 : ch * 512 + 512] for ch in range(2)]

        # ---- feature maps -------------------------------------------
        # Host passes x1/x2 reshaped [128, 2*cols]: partition p holds the
        # channel pair (2p, 2p+1) back-to-back, so each DMA line is one
        # fully contiguous 8-16KB read (vs 128 small strided descriptors).
        # x1 gets the sync queue to itself (Q-proj is the critical path).
        x1_sb = bigbuf.tile([128, 2 * NQ], F16, name="x1_sb")
        x2_sb = bigbuf.tile([128, 2 * N], F16, name="x2_sb")
        nc.sync.dma_start(out=x1_sb, in_=x1)
        nc.scalar.dma_start(out=x2_sb[0:64, :], in_=x2[0:64, :])
        nc.scalar.dma_start(out=x2_sb[64:128, :], in_=x2[64:128, :])

        def x1p(ch, cols):
            return x1_sb[:, ch * NQ + cols.start : ch * NQ + cols.stop]

        def x2p(ch, cols):
            return x2_sb[:, ch * N + cols.start : ch * N + cols.stop]

        q4_sb = bigbuf.tile([128, NQ], F32R, name="q4_sb")
        k4_sb = bigbuf.tile([128, N], F32R, name="k4_sb")
        vt_sb = bigbuf.tile([128, C * N // 128], BF16, name="vt_sb")  # [128, 8192]

        # ---- prep: projections ---------------------------------------
        # Interleaved per 1024-column x2 block so PE work becomes available
        # as each DMA block lands: Q4 first (x1), then per block K4 + V^T.
        with tc.tile_pool(name="prep_psum", bufs=2, space="PSUM") as pp:
            # Q4 [128, 2048] = (Wq stacked 4x) @ x1, then +bq
            psum_q = pp.tile([128, NQ], F32, name="psum_q", tag="prep")
            for t4 in range(NNT):
                cols = slice(t4 * NT, (t4 + 1) * NT)
                for ch in range(2):
                    nc.tensor.matmul(
                        psum_q[:, cols],
                        lhsT=wq4t_sb[ch],
                        rhs=x1p(ch, cols),
                        start=(ch == 0),
                        stop=(ch == 1),
                    )
            nc.vector.tensor_scalar_add(q4_sb, psum_q, bq4_sb)

            for blk in range(4):
                bcols = slice(blk * 1024, (blk + 1) * 1024)
                # K4 for this block
                psum_k = pp.tile([128, 1024], F32, name=f"psum_k{blk}", tag="prep")
                for t2 in range(2):
                    cols = slice(t2 * NT, (t2 + 1) * NT)
                    src_c = slice(blk * 1024 + t2 * NT, blk * 1024 + (t2 + 1) * NT)
                    for ch in range(2):
                        nc.tensor.matmul(
                            psum_k[:, cols],
                            lhsT=wk4t_sb[ch],
                            rhs=x2p(ch, src_c),
                            start=(ch == 0),
                            stop=(ch == 1),
                        )
                nc.vector.tensor_scalar_add(k4_sb[:, bcols], psum_k, bk4_sb)
                # V^T (bf16) for this block's 8 m-chunks
                psum_v = pp.tile([128, 2048], F32, name=f"psum_v{blk}", tag="prep")
                for m8 in range(8):
                    mc = 8 * blk + m8
                    for ch in range(2):
                        nc.tensor.matmul(
                            psum_v[:, m8 * 256 : (m8 + 1) * 256],
                            lhsT=x2p(ch, slice(mc * 128, (mc + 1) * 128)),
                            rhs=wvt_sb[ch],
                            start=(ch == 0),
                            stop=(ch == 1),
                        )
                for h in range(2):
                    cols = slice(h * 1024, (h + 1) * 1024)
                    dst = vt_sb[:, blk * 2048 + h * 1024 : blk * 2048 + (h + 1) * 1024]
                    if h == 0:
                        nc.scalar.copy(dst, psum_v[:, cols])
                    else:
                        nc.vector.tensor_copy(dst, psum_v[:, cols])

        # ---- main attention loop -------------------------------------
        # Flat software pipeline over (nt, sc) steps, each split into two
        # halves h of 2 key-chunks.  The S^T psum is a 2-deep pool of
        # [128, 1024] halves, so exp (ACT) of half (i, h) overlaps the S^T
        # matmuls of the next half/step on the PE instead of serializing.
        # AV/rowsum matmuls of step i are emitted after step i+1's S^T, so
        # the PE always has work while ACT computes exp.
        with (
            tc.tile_pool(name="s_psum", bufs=2, space="PSUM") as sp,
            tc.tile_pool(name="o_psum", bufs=1, space="PSUM") as op,
            tc.tile_pool(name="b_psum", bufs=1, space="PSUM") as bp,
        ):
            state = {}

            def _emit_st_half(nt, sc, h):
                # S^T half: 2 row-packed matmuls (chunks 4*sc+2h+{0,1}) at
                # row-groups {2h, 2h+1}; exp -> bf16 pt on ACT.
                qcols = slice(nt * NT, (nt + 1) * NT)
                psum_s = sp.tile([128, 2 * NT], F32, name=f"ps_{nt}_{sc}_{h}", tag="s")
                for j in range(2):
                    mc = 4 * sc + 2 * h + j
                    rowg = slice(32 * (2 * h + j), 32 * (2 * h + j + 1))
                    nc.tensor.matmul(
                        psum_s[:, j * NT : (j + 1) * NT],
                        lhsT=k4_sb[rowg, mc * 128 : (mc + 1) * 128],
                        rhs=q4_sb[rowg, qcols],
                        start=True,
                        stop=True,
                        tile_position=(32 * (2 * h + j), 0),
                    )
                pt = ptbuf.tile([128, 2 * NT], BF16, name=f"pt_{nt}_{sc}_{h}", tag="pt")
                nc.scalar.activation(out=pt, in_=psum_s, func=AFT.Exp)
                return pt

            def _emit_rs(nt, sc, pt_a, pt_b):
                # col-packed rowsums: 4 concurrent M=1 tiles emitted
                # back-to-back (adjacency is required for the PE to
                # co-execute tile-disjoint matmuls); partials land on
                # partitions {0, 32, 64, 96}.
                if sc == 0:
                    state[nt] = (
                        op.tile([128, NT], F32, name=f"po0_{nt}", tag="o0"),
                        op.tile([128, NT], F32, name=f"po1_{nt}", tag="o1"),
                        op.tile([128, NT], F32, name=f"prs_{nt}", tag="rs"),
                    )
                psum_rs = state[nt][2]
                for g in range(4):
                    h, j = divmod(g, 2)
                    nc.tensor.matmul(
                        psum_rs[32 * g : 32 * (g + 1), :],
                        lhsT=ones_rs,
                        rhs=(pt_a, pt_b)[h][:, j * NT : (j + 1) * NT],
                        start=(sc == 0),
                        stop=(sc == NSC - 1),
                        tile_position=(0, 32 * g),
                        skip_group_check=True,
                    )

            def _emit_av_half(nt, sc, h, pt):
                first, last = (sc == 0 and h == 0), (sc == NSC - 1 and h == 1)
                psum_o0, psum_o1, _ = state[nt]
                for j in range(2):
                    mc = 4 * sc + 2 * h + j
                    pcols = slice(j * NT, (j + 1) * NT)
                    for cc in range(2):
                        nc.tensor.matmul(
                            (psum_o0, psum_o1)[cc],
                            lhsT=vt_sb[
                                :, mc * 256 + cc * 128 : mc * 256 + (cc + 1) * 128
                            ],
                            rhs=pt[:, pcols],
                            start=(first and j == 0),
                            stop=(last and j == 1),
                        )

            def _emit_fin(nt):
                # evacuate PSUM fast (frees banks for the next tile), then
                # normalize on SBUF; everything off the ACT engine so exp
                # never stalls.
                psum_o0, psum_o1, psum_rs = state.pop(nt)
                qcols = slice(nt * NT, (nt + 1) * NT)
                rs_sb = finbuf.tile([128, NT], F32R, name=f"rs_sb_{nt}", tag="rs_sb")
                nc.vector.tensor_copy(rs_sb, psum_rs)
                # GpSimd has no PSUM port: psum evacuation stays on ACT/DVE
                # (one ACT copy per 8-step tile fits in ACT's slack).
                raw0 = finbuf.tile([128, NT], F32, name=f"raw0_{nt}", tag="raw0")
                nc.scalar.copy(raw0, psum_o0)
                raw1 = finbuf.tile([128, NT], F32, name=f"raw1_{nt}", tag="raw1")
                nc.vector.tensor_copy(raw1, psum_o1)
                # gather the 4 partial rows onto adjacent partitions, then a
                # K=4 ones-matmul combines + broadcasts to all 128 partitions
                rs4p = finbuf.tile([4, NT], F32R, name=f"rs4p_{nt}", tag="rs4p")
                nc.sync.dma_start(out=rs4p, in_=rs_sb[0:97:32, :])
                psum_b = bp.tile([128, NT], F32, name=f"pb_{nt}", tag="b")
                nc.tensor.matmul(
                    psum_b, lhsT=ones_bc[0:4, :], rhs=rs4p, start=True, stop=True
                )
                rbc = finbuf.tile([128, NT], F32, name=f"rbc_{nt}", tag="rbc")
                nc.vector.reciprocal_approx_fast(out=rbc, in_=psum_b)
                # keep element-wise work on DVE: GpSimd's software tensor ops
                # measure ~15x slower than DVE on hardware
                for cc, raw in ((0, raw0), (1, raw1)):
                    t_sb = finbuf.tile([128, NT], F32, name=f"t_{nt}_{cc}", tag=f"t{cc}")
                    nc.vector.tensor_mul(t_sb, raw, rbc)
                    o_sb = finbuf.tile([128, NT], F32, name=f"o_{nt}_{cc}", tag=f"o{cc}")
                    nc.vector.tensor_scalar_add(o_sb, t_sb, bv_sb[cc])
                    nc.sync.dma_start(
                        out=out[cc * 128 : (cc + 1) * 128, qcols], in_=o_sb
                    )

            steps = [(nt, sc) for nt in range(NNT) for sc in range(NSC)]
            prev = None
            for nt, sc in steps:
                pt_a = _emit_st_half(nt, sc, 0)
                pt_b = _emit_st_half(nt, sc, 1)
                if prev is not None:
                    pnt, psc, ppa, ppb = prev
                    _emit_rs(pnt, psc, ppa, ppb)
                    _emit_av_half(pnt, psc, 0, ppa)
                    _emit_av_half(pnt, psc, 1, ppb)
                    if psc == NSC - 1:
                        _emit_fin(pnt)
                prev = (nt, sc, pt_a, pt_b)
            pnt, psc, ppa, ppb = prev
            _emit_rs(pnt, psc, ppa, ppb)
            _emit_av_half(pnt, psc, 0, ppa)
            _emit_av_half(pnt, psc, 1, ppb)
            _emit_fin(pnt)


def build_nc():
    nc = bacc.Bacc("TRN2", target_bir_lowering=False, debug=False)
    x1 = nc.dram_tensor("x1", [128, 2 * NQ], F16, kind="ExternalInput").ap()
    x2 = nc.dram_tensor("x2", [128, 2 * N], F16, kind="ExternalInput").ap()
    wall = nc.dram_tensor("wall", [128, 1024], F16, kind="ExternalInput").ap()
    biases = nc.dram_tensor("biases", [128, 4], F32, kind="ExternalInput").ap()
    ones_cd = nc.dram_tensor("ones_c", [128, 32], BF16, kind="ExternalInput").ap()
    ones_fd = nc.dram_tensor("ones_f", [128, 128], F32R, kind="ExternalInput").ap()
    out = nc.dram_tensor("out", [C, NQ], F32, kind="ExternalOutput").ap()
    with tile.TileContext(nc) as tc:
        attn_tile_kernel(tc, out, x1, x2, wall, biases, ones_cd, ones_fd)
    nc.compile()
    return nc


def make_in_maps(f1, f2, Wq, bq, Wk, bk, Wv, bv):
    f1 = np.asarray(f1, dtype=np.float32)
    f2 = np.asarray(f2, dtype=np.float32)
    Wq = np.asarray(Wq, dtype=np.float32)
    Wk = np.asarray(Wk, dtype=np.float32)
    Wv = np.asarray(Wv, dtype=np.float32)
    bq = np.asarray(bq, dtype=np.float32)
    bk = np.asarray(bk, dtype=np.float32)
    bv = np.asarray(bv, dtype=np.float32)

    x1 = f1.reshape(B, C, N).astype(np.float16)
    x2 = f2.reshape(B, C, N).astype(np.float16)
    # channel-pair packing: SBUF partition p holds channels (2p, 2p+1), so
    # weight plane ch = rows (2p+ch) of W^T.  All weights merge into one
    # [128, 1024] fp16 tensor: per plane [wq4 128 | wk4 128 | wv 256].
    wq4t_f = np.concatenate([Wq.T] * 4, axis=1).astype(np.float16)   # [256, 128]
    wk4t_f = np.concatenate([Wk.T] * 4, axis=1).astype(np.float16)
    wvt_f = Wv.T.astype(np.float16)                                  # [256, 256]
    wall = np.ascontiguousarray(
        np.concatenate(
            [
                np.concatenate([wq4t_f[ch::2], wk4t_f[ch::2], wvt_f[ch::2]], axis=1)
                for ch in range(2)
            ],
            axis=1,
        )
    )  # [128, 1024]
    biases = np.ascontiguousarray(
        np.stack(
            [np.tile(bq, 4), np.tile(bk, 4), bv[:128], bv[128:]], axis=1
        ).astype(np.float32)
    )  # [128, 4]
    import ml_dtypes

    ones_c = np.ones((128, 32), ml_dtypes.bfloat16)
    ones_f = np.ones((128, 128), np.float32)

    in_maps = []
    for core in range(8):
        b, h = divmod(core, 2)
        in_maps.append(
            dict(
                # [C, cols] -> [128, 2*cols]: rows (2p, 2p+1) concatenated per
                # partition; a plain reshape since channel rows are adjacent.
                x1=np.ascontiguousarray(x1[b, :, h * NQ : (h + 1) * NQ]).reshape(
                    128, 2 * NQ
                ),
                x2=np.ascontiguousarray(x2[b]).reshape(128, 2 * N),
                wall=wall,
                biases=biases,
                ones_c=ones_c,
                ones_f=ones_f,
            )
        )
    return in_maps


_NC_CACHE = None


def _get_nc():
    global _NC_CACHE
    if _NC_CACHE is None:
        _NC_CACHE = build_nc()
    return _NC_CACHE


def kernel(f1, f2, Wq, bq, Wk, bk, Wv, bv):
    in_maps = make_in_maps(f1, f2, Wq, bq, Wk, bk, Wv, bv)
    res = bass_utils.run_bass_kernel_spmd(_get_nc(), in_maps, core_ids=list(range(8)))
    out = np.empty((B, C, N), np.float32)
    for core in range(8):
        b, h = divmod(core, 2)
        out[b, :, h * NQ : (h + 1) * NQ] = res.results[core]["out"]
    return out.reshape(B, C, 64, 64)
